# revision 25
# baseline (speedup 1.0000x reference)
"""DGAD net (vq_codebook) kernel v4.2 for 8x Trainium2 NeuronCores.

Contract: kernel(**inputs) takes FULL unsharded inputs, returns FULL [4,1]
fp32 output. Batch (128) sharded 16/core; weights replicated; final
all-reduce (sum/128) on host during unshard.

v4.2 vs v4.1 (71.5us):
  - All MLP evacs are single-op ACT Prelu(alpha=.01, scale=) — Prelu is in
    the natural_log_exp_and_others table set together with Copy/Exp/Ln/
    Square, so ONE table load at t0 covers every activation (no switches).
  - cat1/cat2 concat layers folded algebraically: tw1@[s;s-c] ==
    (tw1L+tw1R)@s + bias; cw1@[o;o-proto[cat]] == (cw1L+cw1R)@o + G@onehot.
    Kills catid/ppad/cpad tensors, 4 matmuls, 2 copies from the tail chain.
  - svdd distances via ACT Square(x + (-center)) single op (was sub+mult).
  - pnorm folded into the sim matmul as a 66th contraction row.
  - Small weights in one bf16 blob (f32 LDWEIGHTS was 300-700ns in tail).
  - Bulk DMAs merged to ~0.8-1.5MB pieces (HWDGE ring holds ~3 in flight;
    issue costs ~600ns each); MT/o3s3/blobs ride the slow ACT ring.
"""

import numpy as np
import ml_dtypes

N_CORES = 8
B = 128
BC = B // N_CORES  # 16 samples per core

BF = ml_dtypes.bfloat16
F8 = ml_dtypes.float8_e4m3
WSCALE = 256.0  # fp8 weights stored *256; 1/256 folded into consumer scales

_CACHE = {}


def _build_program():
    import concourse.bass as bass  # noqa: F401
    import concourse.mybir as mybir
    import concourse.tile as tile
    from concourse import bacc
    from contextlib import ExitStack

    dt = mybir.dt
    AF = mybir.ActivationFunctionType
    ALU = mybir.AluOpType
    AX = mybir.AxisListType
    f32, bf16, f8 = dt.float32, dt.bfloat16, dt.float8e4
    INV = 1.0 / WSCALE
    INV2 = INV * INV

    from concourse.hw_specs import get_activation_tables
    _act_set_id = list(get_activation_tables("gen3")).index("natural_log_exp_and_others")

    nc = bacc.Bacc("TRN2", target_bir_lowering=False, debug=False,
                   enable_asserts=True, num_devices=N_CORES)

    def din(name, shape, d):
        return nc.dram_tensor(name, shape, d, kind="ExternalInput").ap()

    xmV_d = din("xmV", [128, 16, 784], f8)    # ch 0-127, [c,b,hw] (DVE)
    xmA_d = din("xmA", [128, 16, 784], f8)    # ch 128-255, [c,b,hw] (ACT)
    xmP_d = din("xmP", [128, 6, 4096], f8)    # ch 256-511 hw<768, [hw%128, hw//128, (ct,b,c_lo)]
    xmP6_d = din("xmP6", [16, 4096], f8)      # hw 768-783 tail
    xdV_d = din("xdV", [128, 8, 16, 49], f8)  # b0-7, [d%128, b, d//128, hw] (DVE)
    xdP_d = din("xdP", [98, 8192], f8)        # b8-15 2-packed, [hw(+49*par), j*2048+d] (PE)
    ow1T_d = din("ow1T", [128, 16, 1024], f8)   # (k p) o -> p k o, *256
    MT_d = din("MT", [128, 4, 1024], f8)        # (wsh.T @ sw1.T)*256, pre-permuted
    ow2T_d = din("ow2T", [128, 8, 512], f8)
    sw2T_d = din("sw2T", [128, 8, 512], f8)
    o3s3_d = din("o3s3", [128, 4, 128], f8)     # cols 0:64 ow3T, 64:128 sw3T
    # blob64 cols: tw2|cw2|qw1|qw2|W1t|W2t|protoT|neg_center|bias_t1
    blob64_d = din("blob64", [64, 390], bf16)
    GT_d = din("GT", [4, 64], bf16)             # (-cw1R @ proto.T).T
    id16_d = din("id16", [16, 16], f32)
    protoF_d = din("protoF", [64, 4], f32)
    ones2_d = din("ones2", [98, 2], bf16)       # [:49]=[1,0], [49:]=[0,1]
    onescol_d = din("onescol", [128, 1], bf16)
    out_d = nc.dram_tensor("out", [1, 4], f32, kind="ExternalOutput").ap()

    with tile.TileContext(nc) as tc, ExitStack() as ctx:
        wp = ctx.enter_context(tc.tile_pool(name="wp", bufs=1))
        xp = ctx.enter_context(tc.tile_pool(name="xp", bufs=1))
        ap = ctx.enter_context(tc.tile_pool(name="ap", bufs=1))
        pmp = ctx.enter_context(tc.tile_pool(name="pmp", bufs=1, space="PSUM"))
        pdp = ctx.enter_context(tc.tile_pool(name="pdp", bufs=1, space="PSUM"))
        pbig = ctx.enter_context(tc.tile_pool(name="pbig", bufs=2, space="PSUM"))
        pt = ctx.enter_context(tc.tile_pool(name="pt", bufs=3, space="PSUM"))
        pc = ctx.enter_context(tc.tile_pool(name="pc", bufs=1, space="PSUM"))

        # ---------- tiles ----------
        ow1_t = wp.tile([128, 16, 1024], f8, tag="ow1")
        MT_t = wp.tile([128, 4, 1024], f8, tag="MT")
        ow2_t = wp.tile([128, 8, 512], f8, tag="ow2")
        sw2_t = wp.tile([128, 8, 512], f8, tag="sw2")
        o3s3_t = wp.tile([128, 4, 128], f8, tag="o3s3")
        blob64_t = wp.tile([64, 390], bf16, tag="blob64")
        GT_t = wp.tile([4, 64], bf16, tag="GT")
        id16_t = wp.tile([16, 16], f32, tag="id16")
        protoF_t = wp.tile([64, 4], f32, tag="protoF")
        ones2_t = wp.tile([98, 2], bf16, tag="ones2")
        onescol_t = wp.tile([128, 1], bf16, tag="onescol")

        xmV_t = xp.tile([128, 16, 784], f8, tag="xmV")
        xmA_t = xp.tile([128, 16, 784], f8, tag="xmA")
        xmP_t = xp.tile([128, 6, 4096], f8, tag="xmP")
        xmP6_t = xp.tile([16, 4096], f8, tag="xmP6")
        xdV_t = xp.tile([128, 8, 16, 49], f8, tag="xdV")
        xdP_t = xp.tile([98, 8192], f8, tag="xdP")

        tw2 = blob64_t[:, 0:64]
        cw2 = blob64_t[:, 64:128]
        qw1 = blob64_t[:, 128:192]
        qw2 = blob64_t[:, 192:256]
        W1t = blob64_t[:, 256:320]
        W2t = blob64_t[:, 320:384]
        protoT = blob64_t[:, 384:388]
        neg_cc = blob64_t[:, 388:389]
        bias_t1 = blob64_t[:, 389:390]

        # ---------- ACT: one table load covers Prelu/Copy/Exp/Ln/Square ----
        ldset = mybir.InstLoadActFuncSet(
            name=f"I-{nc.next_id()}", act_func_set_id=_act_set_id, ins=[], outs=[])
        ldset.engine = mybir.EngineType.Activation
        nc.scalar.add_instruction(ldset)

        # ---------- DMA issue: ACT ring (slow; tiny + late-needed only) -----
        for t_, d_ in ((ones2_t, ones2_d), (onescol_t, onescol_d),
                       (protoF_t, protoF_d), (blob64_t, blob64_d),
                       (o3s3_t, o3s3_d), (id16_t, id16_d), (GT_t, GT_d)):
            nc.scalar.dma_start(out=t_[:], in_=d_)
        nc.scalar.dma_start(out=MT_t[:], in_=MT_d)

        # ---------- DMA issue: Sync ring, consumption-ordered ----------
        nc.sync.dma_start(out=xdV_t[:], in_=xdV_d)
        nc.sync.dma_start(out=xdP_t[:], in_=xdP_d)
        nc.sync.dma_start(out=ow1_t[:, 0:8, :], in_=ow1T_d[:, 0:8, :])
        nc.sync.dma_start(out=xmA_t[:, 0:8, :], in_=xmA_d[:, 0:8, :])
        nc.sync.dma_start(out=ow1_t[:, 8:16, :], in_=ow1T_d[:, 8:16, :])
        nc.sync.dma_start(out=ow2_t[:], in_=ow2T_d)
        nc.sync.dma_start(out=xmV_t[:, 0:8, :], in_=xmV_d[:, 0:8, :])
        nc.sync.dma_start(out=xmA_t[:, 8:16, :], in_=xmA_d[:, 8:16, :])
        nc.sync.dma_start(out=xmP_t[:, 0:2, :], in_=xmP_d[:, 0:2, :])
        nc.sync.dma_start(out=xmV_t[:, 8:16, :], in_=xmV_d[:, 8:16, :])
        nc.sync.dma_start(out=xmP_t[:, 2:4, :], in_=xmP_d[:, 2:4, :])
        nc.sync.dma_start(out=xmP_t[:, 4:5, :], in_=xmP_d[:, 4:5, :])
        nc.sync.dma_start(out=xmP_t[:, 5:6, :], in_=xmP_d[:, 5:6, :])
        nc.sync.dma_start(out=xmP6_t[:], in_=xmP6_d)
        nc.sync.dma_start(out=sw2_t[:], in_=sw2T_d)

        # ---------- gpsimd consts ----------
        ones64 = ap.tile([64, 1], f32, tag="ones64")
        nc.gpsimd.memset(ones64[:], 1.0)
        ones16 = ap.tile([16, 1], f32, tag="ones16")
        nc.gpsimd.memset(ones16[:], 1.0)
        rhs_sim = ap.tile([65, 4], f32, tag="rhs_sim")
        sim_lhs = ap.tile([65, 16], f32, tag="sim_lhs")
        nc.gpsimd.memset(sim_lhs[64:65, :], 1.0)

        # ---------- sbuf activation tiles ----------
        pooled_v = ap.tile([128, 16], f32, tag="pooled_v")
        pooled_a = ap.tile([128, 16], f32, tag="pooled_a")
        pooled_dv = ap.tile([128, 8, 16], f32, tag="pooled_dv")
        scratch = ap.tile([128, 784], bf16, tag="scratch")
        xdb = ap.tile([128, 8, 2, 16], bf16, tag="xdb")   # b = 2j+s
        xmb = ap.tile([128, 16, 4], bf16, tag="xmb")
        y1o = ap.tile([128, 8, 16], bf16, tag="y1o")
        y2o = ap.tile([128, 4, 16], bf16, tag="y2o")
        origin = ap.tile([64, 16], bf16, tag="origin")
        q1 = ap.tile([64, 16], bf16, tag="q1")
        qf = ap.tile([64, 16], bf16, tag="qf")
        osq = ap.tile([64, 16], f32, tag="osq")
        osvdd = ap.tile([1, 16], f32, tag="osvdd")
        y1s = ap.tile([128, 8, 16], bf16, tag="y1s")
        y2s = ap.tile([128, 4, 16], bf16, tag="y2s")
        shallow = ap.tile([64, 16], bf16, tag="shallow")
        t1 = ap.tile([64, 16], bf16, tag="t1")
        t2 = ap.tile([64, 16], f32, tag="t2")
        sim_sb = ap.tile([16, 4], f32, tag="sim_sb")
        m16 = ap.tile([16, 1], f32, tag="m16")
        negm = ap.tile([16, 1], f32, tag="negm")
        onehotT = ap.tile([16, 4], f32, tag="onehotT")
        oh_sb = ap.tile([4, 16], bf16, tag="oh_sb")
        c1 = ap.tile([64, 16], bf16, tag="c1")
        cf = ap.tile([64, 16], bf16, tag="cf")
        csq = ap.tile([64, 16], f32, tag="csq")
        csvdd = ap.tile([1, 16], f32, tag="csvdd")
        al = ap.tile([1, 16], f32, tag="al")
        pT2 = ap.tile([64, 4], f32, tag="pT2")
        e_t = ap.tile([16, 4], f32, tag="e_t")
        s16 = ap.tile([16, 1], f32, tag="s16")
        ce_col = ap.tile([16, 1], f32, tag="ce_col")
        outv = ap.tile([1, 4], f32, tag="outv")

        pool_m = pmp.tile([128, 32], f32, tag="pool_m")       # col = ct*16+b
        pool_d = pdp.tile([128, 4, 16, 2], f32, tag="pool_d")  # [d%128, j, dc, s]

        def prelu(dst, src, scale=None, bias=None):
            kw = {}
            if scale is not None:
                kw["scale"] = scale
            if bias is not None:
                kw["bias"] = bias
            return nc.scalar.activation(dst, src, AF.Prelu, alpha=0.01, **kw)

        # ---------- PE warm-up spin ----------
        warm_ps = pt.tile([128, 16], f32, tag="tail")
        for _ in range(12):
            nc.tensor.matmul(warm_ps[0:2, 0:2], ones2_t[:], ones2_t[:],
                             start=True, stop=True)

        # ---------- PE: x_deep b8-15 pool ----------
        for t in range(64):  # t = j*16 + dc
            nc.tensor.matmul(pool_d[:, t // 16, t % 16, :],
                             xdP_t[:, 128 * t:128 * t + 128],
                             ones2_t[:], start=True, stop=True)
        # ---------- DVE: x_deep b0-7 pool + xdb ----------
        for hf in range(2):
            nc.vector.reduce_sum(pooled_dv[:, 4 * hf:4 * hf + 4, :],
                                 xdV_t[:, 4 * hf:4 * hf + 4, :, :], axis=AX.X)
        nc.vector.tensor_scalar(xdb[:, 0:4, :, :], pooled_dv[:], INV / 49.0,
                                None, op0=ALU.mult)
        for s in range(2):
            nc.vector.tensor_scalar(xdb[:, 4:8, s, :], pool_d[:, :, :, s],
                                    INV / 49.0, None, op0=ALU.mult)
        nc.vector.tensor_tensor(pT2[:], protoF_t[:], protoF_t[:], op=ALU.mult)
        nc.vector.tensor_scalar(rhs_sim[0:64, :], protoF_t[:], -2.0, None,
                                op0=ALU.mult)
        pn_ps = pt.tile([128, 16], f32, tag="tail")
        nc.tensor.matmul(pn_ps[0:1, 0:4], ones64[:], pT2[:], start=True, stop=True)
        nc.vector.tensor_copy(rhs_sim[64:65, :], pn_ps[0:1, 0:4])

        # ---------- PE: origin layer 1 (k-outer, one psum bank) ----------
        y1o_ps = pbig.tile([128, 8, 16], f32, tag="big")
        for k in range(16):
            for m in range(8):
                nc.tensor.matmul(y1o_ps[:, m, :],
                                 ow1_t[:, k, 128 * m:128 * m + 128],
                                 xdb[:, :, :, k],
                                 start=(k == 0), stop=(k == 15))

        # ---------- ACT: x_mid ch 128-255 pool (16 lines) ----------
        for b in range(16):
            nc.scalar.activation(scratch[:], xmA_t[:, b, :], AF.Copy,
                                 accum_out=pooled_a[:, b:b + 1])
        nc.scalar.mul(xmb[:, :, 1], pooled_a[:], INV / 784.0)

        # ---------- DVE: x_mid ch 0-127 pools ----------
        nc.vector.reduce_sum(pooled_v[:, 0:4], xmV_t[:, 0:4, :], axis=AX.X)
        prelu(y1o[:], y1o_ps[:])

        def pool_mm(h, ts=range(32)):
            """PE pool batch for xmP hw-tile h (32 MMs, chains over h)."""
            for t in ts:
                if h < 6:
                    nc.tensor.matmul(pool_m[:, t:t + 1],
                                     xmP_t[:, h, 128 * t:128 * t + 128],
                                     onescol_t[:], start=(h == 0), stop=False)
                else:
                    nc.tensor.matmul(pool_m[:, t:t + 1],
                                     xmP6_t[:, 128 * t:128 * t + 128],
                                     onescol_t[0:16, :], start=False, stop=True)

        # ---------- PE: y2o + origin + q chain, interleaved with pools -----
        y2o_ps = pbig.tile([128, 4, 16], f32, tag="big")
        for k in range(8):
            for m in range(4):
                nc.tensor.matmul(y2o_ps[:, m, :],
                                 ow2_t[:, k, 128 * m:128 * m + 128],
                                 y1o[:, k, :], start=(k == 0), stop=(k == 7))
        pool_mm(0)

        nc.vector.reduce_sum(pooled_v[:, 4:8], xmV_t[:, 4:8, :], axis=AX.X)
        prelu(y2o[:], y2o_ps[:])

        origin_ps = pt.tile([128, 16], f32, tag="tail")
        for k in range(4):
            nc.tensor.matmul(origin_ps[0:64, :], o3s3_t[:, k, 0:64],
                             y2o[:, k, :], start=(k == 0), stop=(k == 3))
        pool_mm(1)
        prelu(origin[:], origin_ps[0:64, :], scale=INV2)

        q1_ps = pt.tile([128, 16], f32, tag="tail")
        nc.tensor.matmul(q1_ps[0:64, :], qw1, origin[:], start=True, stop=True)
        pool_mm(2)
        prelu(q1[:], q1_ps[0:64, :])
        q2_ps = pt.tile([128, 16], f32, tag="tail")
        nc.tensor.matmul(q2_ps[0:64, :], qw2, q1[:], start=True, stop=True)
        pool_mm(3)
        prelu(qf[:], q2_ps[0:64, :])
        c1_ps = pc.tile([128, 16], f32, tag="c1ps")
        nc.tensor.matmul(c1_ps[0:64, :], W2t, origin[:], start=True, stop=False)
        nc.vector.reduce_sum(pooled_v[:, 8:12], xmV_t[:, 8:12, :], axis=AX.X)
        nc.scalar.activation(osq[:], qf[:], AF.Square, bias=neg_cc)
        osvdd_ps = pt.tile([128, 16], f32, tag="tail")
        nc.tensor.matmul(osvdd_ps[0:1, :], ones64[:], osq[:], start=True, stop=True)
        pool_mm(4)
        nc.vector.reduce_sum(pooled_v[:, 12:16], xmV_t[:, 12:16, :], axis=AX.X)
        nc.vector.tensor_copy(osvdd[:], osvdd_ps[0:1, :])
        nc.vector.tensor_scalar(xmb[:, :, 0], pooled_v[:], INV / 784.0, None,
                                op0=ALU.mult)
        y1s_ps = pbig.tile([128, 8, 16], f32, tag="big")

        def m_layer(k, start, stop):
            for m in range(8):
                nc.tensor.matmul(y1s_ps[:, m, :],
                                 MT_t[:, k, 128 * m:128 * m + 128],
                                 xmb[:, :, k], start=start, stop=stop)

        m_layer(0, True, False)
        m_layer(1, False, False)
        pool_mm(5)
        pool_mm(6, range(16))
        nc.vector.tensor_scalar(xmb[:, :, 2], pool_m[:, 0:16],
                                INV / 784.0, None, op0=ALU.mult)
        m_layer(2, False, False)
        pool_mm(6, range(16, 32))
        nc.vector.tensor_scalar(xmb[:, :, 3], pool_m[:, 16:32],
                                INV / 784.0, None, op0=ALU.mult)
        m_layer(3, False, True)
        prelu(y1s[:, 0:4, :], y1s_ps[:, 0:4, :])
        prelu(y1s[:, 4:8, :], y1s_ps[:, 4:8, :])
        y2s_ps = pbig.tile([128, 4, 16], f32, tag="big")
        for k in range(8):
            for m in range(4):
                nc.tensor.matmul(y2s_ps[:, m, :],
                                 sw2_t[:, k, 128 * m:128 * m + 128],
                                 y1s[:, k, :], start=(k == 0), stop=(k == 7))
        prelu(y2s[:], y2s_ps[:])
        sh_ps = pt.tile([128, 16], f32, tag="tail")
        for k in range(4):
            nc.tensor.matmul(sh_ps[0:64, :], o3s3_t[:, k, 64:128],
                             y2s[:, k, :], start=(k == 0), stop=(k == 3))
        prelu(shallow[:], sh_ps[0:64, :], scale=INV2)

        # ---------- texture path (cat1 folded into W1t + bias_t1) ----------
        t1_ps = pt.tile([128, 16], f32, tag="tail")
        nc.tensor.matmul(t1_ps[0:64, :], W1t, shallow[:], start=True, stop=True)
        prelu(t1[:], t1_ps[0:64, :], bias=bias_t1)
        t2_ps = pt.tile([128, 16], f32, tag="tail")
        nc.tensor.matmul(t2_ps[0:64, :], tw2, t1[:], start=True, stop=True)
        prelu(sim_lhs[0:64, :], t2_ps[0:64, :])

        # ---------- sim + argmax + CE (||t||^2 dropped: shift-invariant) --
        sim_ps = pt.tile([128, 16], f32, tag="tail")
        nc.tensor.matmul(sim_ps[0:16, 0:4], sim_lhs[:], rhs_sim[:],
                         start=True, stop=True)
        nc.vector.reduce_max(m16[:], sim_ps[0:16, 0:4], axis=AX.X)
        nc.vector.tensor_scalar(onehotT[:], sim_ps[0:16, 0:4], m16[:, 0:1], None,
                                op0=ALU.is_ge)
        nc.vector.tensor_scalar(negm[:], m16[:], -1.0, None, op0=ALU.mult)
        oh_ps = pt.tile([128, 16], f32, tag="tail")
        nc.tensor.transpose(oh_ps[0:4, 0:16], onehotT[:], id16_t[:])
        nc.vector.tensor_copy(oh_sb[:], oh_ps[0:4, 0:16])

        # ---------- class feat chain (W2t@origin accumulated early) -------
        nc.tensor.matmul(c1_ps[0:64, :], GT_t[:], oh_sb[:], start=False, stop=True)
        prelu(c1[:], c1_ps[0:64, :])
        nc.scalar.activation(e_t[:], sim_ps[0:16, 0:4], AF.Exp, bias=negm[:, 0:1],
                             accum_out=s16[:])
        cw2_ps = pt.tile([128, 16], f32, tag="tail")
        nc.tensor.matmul(cw2_ps[0:64, :], cw2, c1[:], start=True, stop=True)
        prelu(cf[:], cw2_ps[0:64, :])
        nc.scalar.activation(csq[:], cf[:], AF.Square, bias=neg_cc)
        nc.scalar.activation(ce_col[:], s16[:], AF.Ln)
        csvdd_ps = pt.tile([128, 16], f32, tag="tail")
        nc.tensor.matmul(csvdd_ps[0:1, :], ones64[:], csq[:], start=True, stop=True)
        ce_ps = pt.tile([128, 16], f32, tag="tail")
        nc.tensor.matmul(ce_ps[0:1, 0:1], ce_col[:], ones16[:],
                         start=True, stop=True)
        # ---------- align + output ----------
        nc.vector.tensor_tensor(al[:], osvdd[:], csvdd_ps[0:1, :], op=ALU.subtract)
        nc.vector.scalar_tensor_tensor(al[:], al[:], -1.0, al[:],
                                       op0=ALU.mult, op1=ALU.max,
                                       accum_out=outv[0:1, 3:4])
        nc.vector.tensor_copy(outv[0:1, 0:1], ce_ps[0:1, 0:1])
        nc.vector.reduce_sum(outv[0:1, 1:2], osvdd[:], axis=AX.X)
        nc.vector.reduce_sum(outv[0:1, 2:3], csvdd_ps[0:1, :], axis=AX.X)

        nc.sync.dma_start(out=out_d[:], in_=outv[:])

    nc.compile()
    return nc


def _host_prep(inputs):
    f = np.float32
    xm8 = np.asarray(inputs["x_mid"], f).reshape(B, 512, 784).astype(F8)
    xd8 = np.asarray(inputs["x_deep"], f).reshape(B, 2048, 49).astype(F8)

    def T(w):
        return np.ascontiguousarray(np.asarray(w, f).T)

    def T8(w):
        return (T(w) * WSCALE).astype(F8)

    def ptile(w, kk):  # [K, O] -> [128, kk, O] with row k*128+p -> [p, k, :]
        K, O = w.shape
        return np.ascontiguousarray(w.reshape(kk, 128, O).transpose(1, 0, 2))

    M = np.asarray(inputs["w_shallow"], f).T @ np.asarray(inputs["sw1"], f).T

    center = np.asarray(inputs["center"], f)
    proto = np.asarray(inputs["proto"], f)
    tw1 = np.asarray(inputs["tw1"], f)   # [64, 128]
    cw1 = np.asarray(inputs["cw1"], f)   # [64, 128]
    W1 = tw1[:, 0:64] + tw1[:, 64:128]   # t1 = W1 @ shallow + bias_t1
    bias_t1 = -(tw1[:, 64:128] @ center)  # [64]
    W2 = cw1[:, 0:64] + cw1[:, 64:128]   # c1 = W2 @ origin + G @ onehot
    G = -(cw1[:, 64:128] @ proto.T)      # [64, 4]
    ones2 = np.zeros((98, 2), dtype=BF)
    ones2[0:49, 0] = 1
    ones2[49:98, 1] = 1
    o3s3 = np.concatenate([ptile(T8(inputs["ow3"]), 4),
                           ptile(T8(inputs["sw3"]), 4)], axis=2)
    blob64 = np.concatenate(
        [T(inputs["tw2"]), T(inputs["cw2"]), T(inputs["qw1"]),
         T(inputs["qw2"]), T(W1), T(W2), T(proto),
         -center.reshape(64, 1), bias_t1.reshape(64, 1)],
        axis=1).astype(BF)

    shared = {
        "ow1T": ptile(T8(inputs["ow1"]), 16),
        "MT": ptile((M * WSCALE).astype(F8), 4),
        "ow2T": ptile(T8(inputs["ow2"]), 8),
        "sw2T": ptile(T8(inputs["sw2"]), 8),
        "o3s3": np.ascontiguousarray(o3s3),
        "blob64": np.ascontiguousarray(blob64),
        "GT": np.ascontiguousarray(T(G).astype(BF)),
        "id16": np.eye(16, dtype=f),
        "protoF": np.ascontiguousarray(T(proto)),
        "ones2": ones2,
        "onescol": np.ones((128, 1), dtype=BF),
    }
    in_maps = []
    for c in range(N_CORES):
        m = dict(shared)
        xc = xm8[c * BC:(c + 1) * BC]          # [16, 512, 784]
        xdc = xd8[c * BC:(c + 1) * BC]         # [16, 2048, 49]
        m["xmV"] = np.ascontiguousarray(xc[:, 0:128].transpose(1, 0, 2))
        m["xmA"] = np.ascontiguousarray(xc[:, 128:256].transpose(1, 0, 2))
        # xmP: [hw, (ct, b, c_lo)] -> [hw%128, hw//128, 4096] for hw<768
        xp_ = xc[:, 256:512].reshape(16, 2, 128, 784).transpose(3, 1, 0, 2) \
            .reshape(784, 4096)
        m["xmP"] = np.ascontiguousarray(
            xp_[0:768].reshape(6, 128, 4096).transpose(1, 0, 2))
        m["xmP6"] = np.ascontiguousarray(xp_[768:784])
        # xdV: b0-7, [d%128, b, d//128, hw]
        m["xdV"] = np.ascontiguousarray(
            xdc[0:8].reshape(8, 16, 128, 49).transpose(2, 0, 1, 3))
        # xdP: b8-15 packed 2 samples per partition set (even b upper, odd lower)
        hi = xdc[8:16]                          # [8, 2048, 49]
        ev = hi[0::2].transpose(2, 0, 1)        # [49, 4, 2048]
        od = hi[1::2].transpose(2, 0, 1)
        m["xdP"] = np.ascontiguousarray(
            np.concatenate([ev, od], axis=0)).reshape(98, 8192)
        in_maps.append(m)
    return in_maps


def _get_program():
    if "nc" not in _CACHE:
        _CACHE["nc"] = _build_program()
    return _CACHE["nc"]


def _combine(parts):
    tot = np.sum([np.asarray(p, np.float64).ravel() for p in parts], axis=0)
    return (tot / B).astype(np.float32).reshape(4, 1)


def _run(inputs, trace=False):
    from concourse.bass_utils import run_bass_kernel_spmd
    nc = _get_program()
    in_maps = _host_prep(inputs)
    kw = {}
    if trace:
        kw = dict(trace=True, trace_cores=list(range(N_CORES)))
    res = run_bass_kernel_spmd(nc, in_maps, list(range(N_CORES)), **kw)
    out = _combine([res.results[i]["out"] for i in range(N_CORES)])
    return out, res


def kernel(**inputs):
    out, _ = _run(inputs, trace=False)
    return out


def kernel_traced(**inputs):
    """Returns (output, exec_time_ns) using the NTFF profile (max over cores)."""
    out, res = _run(inputs, trace=True)
    return out, res.exec_time_ns


# revision 26
# speedup vs baseline: 1.0174x; 1.0174x over previous
"""DGAD net (vq_codebook) kernel v4.2 for 8x Trainium2 NeuronCores.

Contract: kernel(**inputs) takes FULL unsharded inputs, returns FULL [4,1]
fp32 output. Batch (128) sharded 16/core; weights replicated; final
all-reduce (sum/128) on host during unshard.

v4.2 vs v4.1 (71.5us):
  - All MLP evacs are single-op ACT Prelu(alpha=.01, scale=) — Prelu is in
    the natural_log_exp_and_others table set together with Copy/Exp/Ln/
    Square, so ONE table load at t0 covers every activation (no switches).
  - cat1/cat2 concat layers folded algebraically: tw1@[s;s-c] ==
    (tw1L+tw1R)@s + bias; cw1@[o;o-proto[cat]] == (cw1L+cw1R)@o + G@onehot.
    Kills catid/ppad/cpad tensors, 4 matmuls, 2 copies from the tail chain.
  - svdd distances via ACT Square(x + (-center)) single op (was sub+mult).
  - pnorm folded into the sim matmul as a 66th contraction row.
  - Small weights in one bf16 blob (f32 LDWEIGHTS was 300-700ns in tail).
  - Bulk DMAs merged to ~0.8-1.5MB pieces (HWDGE ring holds ~3 in flight;
    issue costs ~600ns each); MT/o3s3/blobs ride the slow ACT ring.
"""

import numpy as np
import ml_dtypes

N_CORES = 8
B = 128
BC = B // N_CORES  # 16 samples per core

BF = ml_dtypes.bfloat16
F8 = ml_dtypes.float8_e4m3
WSCALE = 256.0  # fp8 weights stored *256; 1/256 folded into consumer scales

_CACHE = {}


def _build_program():
    import concourse.bass as bass  # noqa: F401
    import concourse.mybir as mybir
    import concourse.tile as tile
    from concourse import bacc
    from contextlib import ExitStack

    dt = mybir.dt
    AF = mybir.ActivationFunctionType
    ALU = mybir.AluOpType
    AX = mybir.AxisListType
    f32, bf16, f8 = dt.float32, dt.bfloat16, dt.float8e4
    INV = 1.0 / WSCALE
    INV2 = INV * INV

    from concourse.hw_specs import get_activation_tables
    _act_set_id = list(get_activation_tables("gen3")).index("natural_log_exp_and_others")

    nc = bacc.Bacc("TRN2", target_bir_lowering=False, debug=False,
                   enable_asserts=True, num_devices=N_CORES)

    def din(name, shape, d):
        return nc.dram_tensor(name, shape, d, kind="ExternalInput").ap()

    xmV_d = din("xmV", [128, 16, 784], f8)    # ch 0-127, [c,b,hw] (DVE)
    xmA_d = din("xmA", [128, 16, 784], f8)    # ch 128-255, [c,b,hw] (ACT)
    xmP_d = din("xmP", [128, 6, 4096], f8)    # ch 256-511 hw<768, [hw%128, hw//128, (ct,b,c_lo)]
    xmP6_d = din("xmP6", [16, 4096], f8)      # hw 768-783 tail
    xdV_d = din("xdV", [128, 8, 16, 49], f8)  # b0-7, [d%128, b, d//128, hw] (DVE)
    xdP_d = din("xdP", [98, 8192], f8)        # b8-15 2-packed, [hw(+49*par), j*2048+d] (PE)
    ow1T_d = din("ow1T", [128, 16, 1024], f8)   # (k p) o -> p k o, *256
    MT_d = din("MT", [128, 4, 1024], f8)        # (wsh.T @ sw1.T)*256, pre-permuted
    ow2T_d = din("ow2T", [128, 8, 512], f8)
    sw2T_d = din("sw2T", [128, 8, 512], f8)
    o3s3_d = din("o3s3", [128, 4, 128], f8)     # cols 0:64 ow3T, 64:128 sw3T
    # blob64 cols: tw2|cw2|qw1|qw2|W1t|W2t|protoT|neg_center|bias_t1
    blob64_d = din("blob64", [64, 390], bf16)
    GT_d = din("GT", [4, 64], bf16)             # (-cw1R @ proto.T).T
    id16_d = din("id16", [16, 16], f32)
    protoF_d = din("protoF", [64, 4], f32)
    ones2_d = din("ones2", [98, 2], bf16)       # [:49]=[1,0], [49:]=[0,1]
    onescol_d = din("onescol", [128, 1], bf16)
    out_d = nc.dram_tensor("out", [1, 4], f32, kind="ExternalOutput").ap()

    with tile.TileContext(nc) as tc, ExitStack() as ctx:
        wp = ctx.enter_context(tc.tile_pool(name="wp", bufs=1))
        xp = ctx.enter_context(tc.tile_pool(name="xp", bufs=1))
        ap = ctx.enter_context(tc.tile_pool(name="ap", bufs=1))
        pmp = ctx.enter_context(tc.tile_pool(name="pmp", bufs=1, space="PSUM"))
        pdp = ctx.enter_context(tc.tile_pool(name="pdp", bufs=1, space="PSUM"))
        pbig = ctx.enter_context(tc.tile_pool(name="pbig", bufs=2, space="PSUM"))
        pt = ctx.enter_context(tc.tile_pool(name="pt", bufs=3, space="PSUM"))
        pc = ctx.enter_context(tc.tile_pool(name="pc", bufs=1, space="PSUM"))

        # ---------- tiles ----------
        ow1_t = wp.tile([128, 16, 1024], f8, tag="ow1")
        MT_t = wp.tile([128, 4, 1024], f8, tag="MT")
        ow2_t = wp.tile([128, 8, 512], f8, tag="ow2")
        sw2_t = wp.tile([128, 8, 512], f8, tag="sw2")
        o3s3_t = wp.tile([128, 4, 128], f8, tag="o3s3")
        blob64_t = wp.tile([64, 390], bf16, tag="blob64")
        GT_t = wp.tile([4, 64], bf16, tag="GT")
        id16_t = wp.tile([16, 16], f32, tag="id16")
        protoF_t = wp.tile([64, 4], f32, tag="protoF")
        ones2_t = wp.tile([98, 2], bf16, tag="ones2")
        onescol_t = wp.tile([128, 1], bf16, tag="onescol")

        xmV_t = xp.tile([128, 16, 784], f8, tag="xmV")
        xmA_t = xp.tile([128, 16, 784], f8, tag="xmA")
        xmP_t = xp.tile([128, 6, 4096], f8, tag="xmP")
        xmP6_t = xp.tile([16, 4096], f8, tag="xmP6")
        xdV_t = xp.tile([128, 8, 16, 49], f8, tag="xdV")
        xdP_t = xp.tile([98, 8192], f8, tag="xdP")

        tw2 = blob64_t[:, 0:64]
        cw2 = blob64_t[:, 64:128]
        qw1 = blob64_t[:, 128:192]
        qw2 = blob64_t[:, 192:256]
        W1t = blob64_t[:, 256:320]
        W2t = blob64_t[:, 320:384]
        protoT = blob64_t[:, 384:388]
        neg_cc = blob64_t[:, 388:389]
        bias_t1 = blob64_t[:, 389:390]

        # ---------- ACT: one table load covers Prelu/Copy/Exp/Ln/Square ----
        ldset = mybir.InstLoadActFuncSet(
            name=f"I-{nc.next_id()}", act_func_set_id=_act_set_id, ins=[], outs=[])
        ldset.engine = mybir.EngineType.Activation
        nc.scalar.add_instruction(ldset)

        # ---------- DMA issue: ACT ring (slow; tiny + late-needed only) -----
        for t_, d_ in ((ones2_t, ones2_d), (onescol_t, onescol_d),
                       (protoF_t, protoF_d), (blob64_t, blob64_d),
                       (o3s3_t, o3s3_d), (id16_t, id16_d), (GT_t, GT_d)):
            nc.scalar.dma_start(out=t_[:], in_=d_)
        nc.scalar.dma_start(out=MT_t[:], in_=MT_d)

        # ---------- DMA issue: Sync ring, consumption-ordered ----------
        nc.sync.dma_start(out=xdV_t[:], in_=xdV_d)
        nc.sync.dma_start(out=xdP_t[:], in_=xdP_d)
        nc.sync.dma_start(out=ow1_t[:, 0:8, :], in_=ow1T_d[:, 0:8, :])
        nc.sync.dma_start(out=xmA_t[:, 0:8, :], in_=xmA_d[:, 0:8, :])
        nc.sync.dma_start(out=ow1_t[:, 8:16, :], in_=ow1T_d[:, 8:16, :])
        nc.sync.dma_start(out=ow2_t[:], in_=ow2T_d)
        nc.sync.dma_start(out=xmV_t[:, 0:8, :], in_=xmV_d[:, 0:8, :])
        nc.sync.dma_start(out=xmA_t[:, 8:16, :], in_=xmA_d[:, 8:16, :])
        nc.sync.dma_start(out=xmP_t[:, 0:2, :], in_=xmP_d[:, 0:2, :])
        nc.sync.dma_start(out=xmV_t[:, 8:16, :], in_=xmV_d[:, 8:16, :])
        nc.sync.dma_start(out=xmP_t[:, 2:4, :], in_=xmP_d[:, 2:4, :])
        nc.sync.dma_start(out=xmP_t[:, 4:5, :], in_=xmP_d[:, 4:5, :])
        nc.sync.dma_start(out=xmP_t[:, 5:6, :], in_=xmP_d[:, 5:6, :])
        nc.sync.dma_start(out=xmP6_t[:], in_=xmP6_d)
        nc.sync.dma_start(out=sw2_t[:], in_=sw2T_d)

        # ---------- gpsimd consts ----------
        ones64 = ap.tile([64, 1], f32, tag="ones64")
        nc.gpsimd.memset(ones64[:], 1.0)
        ones16 = ap.tile([16, 1], f32, tag="ones16")
        nc.gpsimd.memset(ones16[:], 1.0)
        rhs_sim = ap.tile([65, 4], f32, tag="rhs_sim")
        sim_lhs = ap.tile([65, 16], f32, tag="sim_lhs")
        nc.gpsimd.memset(sim_lhs[64:65, :], 1.0)

        # ---------- sbuf activation tiles ----------
        pooled_v = ap.tile([128, 16], f32, tag="pooled_v")
        pooled_a = ap.tile([128, 16], f32, tag="pooled_a")
        pooled_dv = ap.tile([128, 8, 16], f32, tag="pooled_dv")
        scratch = ap.tile([128, 784], bf16, tag="scratch")
        xdb = ap.tile([128, 8, 2, 16], bf16, tag="xdb")   # b = 2j+s
        xmb = ap.tile([128, 16, 4], bf16, tag="xmb")
        y1o = ap.tile([128, 8, 16], bf16, tag="y1o")
        y2o = ap.tile([128, 4, 16], bf16, tag="y2o")
        origin = ap.tile([64, 16], bf16, tag="origin")
        q1 = ap.tile([64, 16], bf16, tag="q1")
        qf = ap.tile([64, 16], bf16, tag="qf")
        osq = ap.tile([64, 16], f32, tag="osq")
        osvdd = ap.tile([1, 16], f32, tag="osvdd")
        y1s = ap.tile([128, 8, 16], bf16, tag="y1s")
        y2s = ap.tile([128, 4, 16], bf16, tag="y2s")
        shallow = ap.tile([64, 16], bf16, tag="shallow")
        t1 = ap.tile([64, 16], bf16, tag="t1")
        t2 = ap.tile([64, 16], f32, tag="t2")
        sim_sb = ap.tile([16, 4], f32, tag="sim_sb")
        m16 = ap.tile([16, 1], f32, tag="m16")
        negm = ap.tile([16, 1], f32, tag="negm")
        onehotT = ap.tile([16, 4], f32, tag="onehotT")
        oh_sb = ap.tile([4, 16], bf16, tag="oh_sb")
        c1 = ap.tile([64, 16], bf16, tag="c1")
        cf = ap.tile([64, 16], bf16, tag="cf")
        csq = ap.tile([64, 16], f32, tag="csq")
        csvdd = ap.tile([1, 16], f32, tag="csvdd")
        al = ap.tile([1, 16], f32, tag="al")
        pT2 = ap.tile([64, 4], f32, tag="pT2")
        e_t = ap.tile([16, 4], f32, tag="e_t")
        s16 = ap.tile([16, 1], f32, tag="s16")
        ce_col = ap.tile([16, 1], f32, tag="ce_col")
        outv = ap.tile([1, 4], f32, tag="outv")

        pool_m = pmp.tile([128, 32], f32, tag="pool_m")       # col = ct*16+b
        pool_d = pdp.tile([128, 4, 16, 2], f32, tag="pool_d")  # [d%128, j, dc, s]

        def prelu(dst, src, scale=None, bias=None):
            kw = {}
            if scale is not None:
                kw["scale"] = scale
            if bias is not None:
                kw["bias"] = bias
            return nc.scalar.activation(dst, src, AF.Prelu, alpha=0.01, **kw)

        # ---------- PE warm-up spin ----------
        warm_ps = pt.tile([128, 16], f32, tag="tail")
        for _ in range(12):
            nc.tensor.matmul(warm_ps[0:2, 0:2], ones2_t[:], ones2_t[:],
                             start=True, stop=True)

        # ---------- PE: x_deep b8-15 pool ----------
        for t in range(64):  # t = j*16 + dc
            nc.tensor.matmul(pool_d[:, t // 16, t % 16, :],
                             xdP_t[:, 128 * t:128 * t + 128],
                             ones2_t[:], start=True, stop=True)
        # ---------- DVE: x_deep b0-7 pool + xdb ----------
        for hf in range(2):
            nc.vector.reduce_sum(pooled_dv[:, 4 * hf:4 * hf + 4, :],
                                 xdV_t[:, 4 * hf:4 * hf + 4, :, :], axis=AX.X)
        nc.vector.tensor_scalar(xdb[:, 0:4, :, :], pooled_dv[:], INV / 49.0,
                                None, op0=ALU.mult)
        for s in range(2):
            nc.vector.tensor_scalar(xdb[:, 4:8, s, :], pool_d[:, :, :, s],
                                    INV / 49.0, None, op0=ALU.mult)
        nc.vector.tensor_tensor(pT2[:], protoF_t[:], protoF_t[:], op=ALU.mult)
        nc.vector.tensor_scalar(rhs_sim[0:64, :], protoF_t[:], -2.0, None,
                                op0=ALU.mult)
        pn_ps = pt.tile([128, 16], f32, tag="tail")
        nc.tensor.matmul(pn_ps[0:1, 0:4], ones64[:], pT2[:], start=True, stop=True)
        nc.vector.tensor_copy(rhs_sim[64:65, :], pn_ps[0:1, 0:4])

        # ---------- PE: origin layer 1 (k-outer, one psum bank) ----------
        y1o_ps = pbig.tile([128, 8, 16], f32, tag="big")
        for k in range(16):
            for m in range(8):
                nc.tensor.matmul(y1o_ps[:, m, :],
                                 ow1_t[:, k, 128 * m:128 * m + 128],
                                 xdb[:, :, :, k],
                                 start=(k == 0), stop=(k == 15))

        # ---------- ACT: x_mid ch 128-255 pool (16 lines) ----------
        for b in range(16):
            nc.scalar.activation(scratch[:], xmA_t[:, b, :], AF.Copy,
                                 accum_out=pooled_a[:, b:b + 1])
        nc.scalar.mul(xmb[:, :, 1], pooled_a[:], INV / 784.0)

        # ---------- DVE: x_mid ch 0-127 pools ----------
        nc.vector.reduce_sum(pooled_v[:, 0:4], xmV_t[:, 0:4, :], axis=AX.X)
        prelu(y1o[:], y1o_ps[:])

        def pool_mm(h, ts=range(32)):
            """PE pool batch for xmP hw-tile h (32 MMs, chains over h)."""
            for t in ts:
                if h < 6:
                    nc.tensor.matmul(pool_m[:, t:t + 1],
                                     xmP_t[:, h, 128 * t:128 * t + 128],
                                     onescol_t[:], start=(h == 0), stop=False)
                else:
                    nc.tensor.matmul(pool_m[:, t:t + 1],
                                     xmP6_t[:, 128 * t:128 * t + 128],
                                     onescol_t[0:16, :], start=False, stop=True)

        # ---------- PE: y2o + origin + q chain, interleaved with pools -----
        y2o_ps = pbig.tile([128, 4, 16], f32, tag="big")
        for k in range(8):
            for m in range(4):
                nc.tensor.matmul(y2o_ps[:, m, :],
                                 ow2_t[:, k, 128 * m:128 * m + 128],
                                 y1o[:, k, :], start=(k == 0), stop=(k == 7))
        pool_mm(0)

        nc.vector.reduce_sum(pooled_v[:, 4:8], xmV_t[:, 4:8, :], axis=AX.X)
        prelu(y2o[:], y2o_ps[:])

        origin_ps = pt.tile([128, 16], f32, tag="tail")
        for k in range(4):
            nc.tensor.matmul(origin_ps[0:64, :], o3s3_t[:, k, 0:64],
                             y2o[:, k, :], start=(k == 0), stop=(k == 3))
        pool_mm(1)
        prelu(origin[:], origin_ps[0:64, :], scale=INV2)

        q1_ps = pt.tile([128, 16], f32, tag="tail")
        nc.tensor.matmul(q1_ps[0:64, :], qw1, origin[:], start=True, stop=True)
        pool_mm(2)
        prelu(q1[:], q1_ps[0:64, :])
        q2_ps = pt.tile([128, 16], f32, tag="tail")
        nc.tensor.matmul(q2_ps[0:64, :], qw2, q1[:], start=True, stop=True)
        pool_mm(3)
        prelu(qf[:], q2_ps[0:64, :])
        c1_ps = pc.tile([128, 16], f32, tag="c1ps")
        nc.tensor.matmul(c1_ps[0:64, :], W2t, origin[:], start=True, stop=False)
        nc.vector.reduce_sum(pooled_v[:, 8:12], xmV_t[:, 8:12, :], axis=AX.X)
        nc.scalar.activation(osq[:], qf[:], AF.Square, bias=neg_cc)
        osvdd_ps = pt.tile([128, 16], f32, tag="tail")
        nc.tensor.matmul(osvdd_ps[0:1, :], ones64[:], osq[:], start=True, stop=True)
        pool_mm(4)
        nc.vector.reduce_sum(pooled_v[:, 12:16], xmV_t[:, 12:16, :], axis=AX.X)
        nc.vector.tensor_copy(osvdd[:], osvdd_ps[0:1, :])
        nc.vector.tensor_scalar(xmb[:, :, 0], pooled_v[:], INV / 784.0, None,
                                op0=ALU.mult)
        y1s_ps = pbig.tile([128, 8, 16], f32, tag="big")

        def m_layer(k, start, stop):
            for m in range(8):
                nc.tensor.matmul(y1s_ps[:, m, :],
                                 MT_t[:, k, 128 * m:128 * m + 128],
                                 xmb[:, :, k], start=start, stop=stop)

        m_layer(0, True, False)
        m_layer(1, False, False)
        pool_mm(5)
        pool_mm(6, range(16))
        nc.vector.tensor_scalar(xmb[:, :, 2], pool_m[:, 0:16],
                                INV / 784.0, None, op0=ALU.mult)
        m_layer(2, False, False)
        pool_mm(6, range(16, 32))
        nc.vector.tensor_scalar(xmb[:, :, 3], pool_m[:, 16:32],
                                INV / 784.0, None, op0=ALU.mult)
        m_layer(3, False, True)
        prelu(y1s[:, 0:4, :], y1s_ps[:, 0:4, :])
        prelu(y1s[:, 4:8, :], y1s_ps[:, 4:8, :])
        y2s_ps = pbig.tile([128, 4, 16], f32, tag="big")
        for k in range(8):
            for m in range(4):
                nc.tensor.matmul(y2s_ps[:, m, :],
                                 sw2_t[:, k, 128 * m:128 * m + 128],
                                 y1s[:, k, :], start=(k == 0), stop=(k == 7))
        prelu(y2s[:, 0:2, :], y2s_ps[:, 0:2, :])
        prelu(y2s[:, 2:4, :], y2s_ps[:, 2:4, :])
        sh_ps = pt.tile([128, 16], f32, tag="tail")
        for k in range(4):
            nc.tensor.matmul(sh_ps[0:64, :], o3s3_t[:, k, 64:128],
                             y2s[:, k, :], start=(k == 0), stop=(k == 3))
        prelu(shallow[:], sh_ps[0:64, :], scale=INV2)

        # ---------- texture path (cat1 folded into W1t + bias_t1) ----------
        t1_ps = pt.tile([128, 16], f32, tag="tail")
        nc.tensor.matmul(t1_ps[0:64, :], W1t, shallow[:], start=True, stop=True)
        prelu(t1[:], t1_ps[0:64, :], bias=bias_t1)
        t2_ps = pt.tile([128, 16], f32, tag="tail")
        nc.tensor.matmul(t2_ps[0:64, :], tw2, t1[:], start=True, stop=True)
        prelu(sim_lhs[0:64, :], t2_ps[0:64, :])

        # ---------- sim + argmax + CE (||t||^2 dropped: shift-invariant) --
        sim_ps = pt.tile([128, 16], f32, tag="tail")
        nc.tensor.matmul(sim_ps[0:16, 0:4], sim_lhs[:], rhs_sim[:],
                         start=True, stop=True)
        nc.vector.reduce_max(m16[:], sim_ps[0:16, 0:4], axis=AX.X)
        nc.vector.tensor_scalar(onehotT[:], sim_ps[0:16, 0:4], m16[:, 0:1], None,
                                op0=ALU.is_ge)
        nc.vector.tensor_scalar(negm[:], m16[:], -1.0, None, op0=ALU.mult)
        oh_ps = pt.tile([128, 16], f32, tag="tail")
        nc.tensor.transpose(oh_ps[0:4, 0:16], onehotT[:], id16_t[:])
        nc.vector.tensor_copy(oh_sb[:], oh_ps[0:4, 0:16])

        # ---------- class feat chain (W2t@origin accumulated early) -------
        nc.tensor.matmul(c1_ps[0:64, :], GT_t[:], oh_sb[:], start=False, stop=True)
        prelu(c1[:], c1_ps[0:64, :])
        nc.scalar.activation(e_t[:], sim_ps[0:16, 0:4], AF.Exp, bias=negm[:, 0:1],
                             accum_out=s16[:])
        cw2_ps = pt.tile([128, 16], f32, tag="tail")
        nc.tensor.matmul(cw2_ps[0:64, :], cw2, c1[:], start=True, stop=True)
        prelu(cf[:], cw2_ps[0:64, :])
        nc.scalar.activation(csq[:], cf[:], AF.Square, bias=neg_cc)
        nc.scalar.activation(ce_col[:], s16[:], AF.Ln)
        csvdd_ps = pt.tile([128, 16], f32, tag="tail")
        nc.tensor.matmul(csvdd_ps[0:1, :], ones64[:], csq[:], start=True, stop=True)
        ce_ps = pt.tile([128, 16], f32, tag="tail")
        nc.tensor.matmul(ce_ps[0:1, 0:1], ce_col[:], ones16[:],
                         start=True, stop=True)
        # ---------- align + output ----------
        nc.vector.tensor_tensor(al[:], osvdd[:], csvdd_ps[0:1, :], op=ALU.subtract)
        nc.vector.scalar_tensor_tensor(al[:], al[:], -1.0, al[:],
                                       op0=ALU.mult, op1=ALU.max,
                                       accum_out=outv[0:1, 3:4])
        nc.vector.tensor_copy(outv[0:1, 0:1], ce_ps[0:1, 0:1])
        nc.vector.reduce_sum(outv[0:1, 1:2], osvdd[:], axis=AX.X)
        nc.vector.reduce_sum(outv[0:1, 2:3], csvdd_ps[0:1, :], axis=AX.X)

        nc.sync.dma_start(out=out_d[:], in_=outv[:])

    nc.compile()
    return nc


def _host_prep(inputs):
    f = np.float32
    xm8 = np.asarray(inputs["x_mid"], f).reshape(B, 512, 784).astype(F8)
    xd8 = np.asarray(inputs["x_deep"], f).reshape(B, 2048, 49).astype(F8)

    def T(w):
        return np.ascontiguousarray(np.asarray(w, f).T)

    def T8(w):
        return (T(w) * WSCALE).astype(F8)

    def ptile(w, kk):  # [K, O] -> [128, kk, O] with row k*128+p -> [p, k, :]
        K, O = w.shape
        return np.ascontiguousarray(w.reshape(kk, 128, O).transpose(1, 0, 2))

    M = np.asarray(inputs["w_shallow"], f).T @ np.asarray(inputs["sw1"], f).T

    center = np.asarray(inputs["center"], f)
    proto = np.asarray(inputs["proto"], f)
    tw1 = np.asarray(inputs["tw1"], f)   # [64, 128]
    cw1 = np.asarray(inputs["cw1"], f)   # [64, 128]
    W1 = tw1[:, 0:64] + tw1[:, 64:128]   # t1 = W1 @ shallow + bias_t1
    bias_t1 = -(tw1[:, 64:128] @ center)  # [64]
    W2 = cw1[:, 0:64] + cw1[:, 64:128]   # c1 = W2 @ origin + G @ onehot
    G = -(cw1[:, 64:128] @ proto.T)      # [64, 4]
    ones2 = np.zeros((98, 2), dtype=BF)
    ones2[0:49, 0] = 1
    ones2[49:98, 1] = 1
    o3s3 = np.concatenate([ptile(T8(inputs["ow3"]), 4),
                           ptile(T8(inputs["sw3"]), 4)], axis=2)
    blob64 = np.concatenate(
        [T(inputs["tw2"]), T(inputs["cw2"]), T(inputs["qw1"]),
         T(inputs["qw2"]), T(W1), T(W2), T(proto),
         -center.reshape(64, 1), bias_t1.reshape(64, 1)],
        axis=1).astype(BF)

    shared = {
        "ow1T": ptile(T8(inputs["ow1"]), 16),
        "MT": ptile((M * WSCALE).astype(F8), 4),
        "ow2T": ptile(T8(inputs["ow2"]), 8),
        "sw2T": ptile(T8(inputs["sw2"]), 8),
        "o3s3": np.ascontiguousarray(o3s3),
        "blob64": np.ascontiguousarray(blob64),
        "GT": np.ascontiguousarray(T(G).astype(BF)),
        "id16": np.eye(16, dtype=f),
        "protoF": np.ascontiguousarray(T(proto)),
        "ones2": ones2,
        "onescol": np.ones((128, 1), dtype=BF),
    }
    in_maps = []
    for c in range(N_CORES):
        m = dict(shared)
        xc = xm8[c * BC:(c + 1) * BC]          # [16, 512, 784]
        xdc = xd8[c * BC:(c + 1) * BC]         # [16, 2048, 49]
        m["xmV"] = np.ascontiguousarray(xc[:, 0:128].transpose(1, 0, 2))
        m["xmA"] = np.ascontiguousarray(xc[:, 128:256].transpose(1, 0, 2))
        # xmP: [hw, (ct, b, c_lo)] -> [hw%128, hw//128, 4096] for hw<768
        xp_ = xc[:, 256:512].reshape(16, 2, 128, 784).transpose(3, 1, 0, 2) \
            .reshape(784, 4096)
        m["xmP"] = np.ascontiguousarray(
            xp_[0:768].reshape(6, 128, 4096).transpose(1, 0, 2))
        m["xmP6"] = np.ascontiguousarray(xp_[768:784])
        # xdV: b0-7, [d%128, b, d//128, hw]
        m["xdV"] = np.ascontiguousarray(
            xdc[0:8].reshape(8, 16, 128, 49).transpose(2, 0, 1, 3))
        # xdP: b8-15 packed 2 samples per partition set (even b upper, odd lower)
        hi = xdc[8:16]                          # [8, 2048, 49]
        ev = hi[0::2].transpose(2, 0, 1)        # [49, 4, 2048]
        od = hi[1::2].transpose(2, 0, 1)
        m["xdP"] = np.ascontiguousarray(
            np.concatenate([ev, od], axis=0)).reshape(98, 8192)
        in_maps.append(m)
    return in_maps


def _get_program():
    if "nc" not in _CACHE:
        _CACHE["nc"] = _build_program()
    return _CACHE["nc"]


def _combine(parts):
    tot = np.sum([np.asarray(p, np.float64).ravel() for p in parts], axis=0)
    return (tot / B).astype(np.float32).reshape(4, 1)


def _run(inputs, trace=False):
    from concourse.bass_utils import run_bass_kernel_spmd
    nc = _get_program()
    in_maps = _host_prep(inputs)
    kw = {}
    if trace:
        kw = dict(trace=True, trace_cores=list(range(N_CORES)))
    res = run_bass_kernel_spmd(nc, in_maps, list(range(N_CORES)), **kw)
    out = _combine([res.results[i]["out"] for i in range(N_CORES)])
    return out, res


def kernel(**inputs):
    out, _ = _run(inputs, trace=False)
    return out


def kernel_traced(**inputs):
    """Returns (output, exec_time_ns) using the NTFF profile (max over cores)."""
    out, res = _run(inputs, trace=True)
    return out, res.exec_time_ns
